# revision 1
# baseline (speedup 1.0000x reference)
"""Causal multi-head attention on 8 TRN2 NeuronCores.

Sharding: tensor-parallel over heads (16 heads -> 2 per core). Each core
computes Q/K/V projections for its 2 heads against the full (replicated)
input, runs causal attention for them, then an AllToAll redistributes the
per-head attention outputs so each core owns a slice of sequence rows and
contracts the full head dimension for the output projection.

All device layouts are "transposed" (feature dim on partitions) so no
on-device transpose is ever needed:
  xT (D, B*S) -> QT/KT (128, S) per batch -> scores^T (s, q) -> P^T ->
  attnT (d_local, q) -> A2A -> y^T (D, rows_slice).

Matmuls run in bf16 (fp32 matmul is 4x slower on the PE array); PSUM
accumulation is fp32. Softmax skips the max-subtraction (score scale makes
exp overflow impossible) and gets the denominator for free from a ones
column appended to V; normalization happens on the small attnT tiles.
"""

import os
import sys

for p in ("/opt/trn_rl_repo", "/root/.axon_site/_ro/trn_rl_repo"):
    if os.path.isdir(p) and p not in sys.path:
        sys.path.append(p)

import numpy as np
import ml_dtypes

import concourse.mybir as mybir
import concourse.tile as tile
from concourse import bacc
from concourse.bass_utils import run_bass_kernel_spmd

B, S, D, H, DK = 4, 2048, 1024, 16, 64
NCORES = 8
HPC = H // NCORES          # heads per core = 2
N = B * S                  # 8192 flattened rows
RPC = N // NCORES          # rows per core after A2A = 1024
BF16 = mybir.dt.bfloat16
F32 = mybir.dt.float32

_cache = {}


def _build(dbg=False):
    nc = bacc.Bacc(None, target_bir_lowering=False, debug=False)

    dbg_taps = {}

    def tap(name, src_ap, shape, dtype):
        if not dbg:
            return
        t = nc.dram_tensor(f"dbg_{name}", shape, dtype, kind="ExternalOutput")
        nc.sync.dma_start(t[:], src_ap)
        dbg_taps[name] = t

    xt_ext = nc.dram_tensor("xt", [D, N], BF16, kind="ExternalInput")
    wq_ext = nc.dram_tensor("wq", [D, 128], BF16, kind="ExternalInput")
    wk_ext = nc.dram_tensor("wk", [D, 128], BF16, kind="ExternalInput")
    wv_ext = nc.dram_tensor("wv", [D, 128], BF16, kind="ExternalInput")
    wo_ext = nc.dram_tensor("wo", [D, D], BF16, kind="ExternalInput")
    mk_ext = nc.dram_tensor("mk", [4, 128, 512], BF16, kind="ExternalInput")
    eye_ext = nc.dram_tensor("eye", [128, 128], BF16, kind="ExternalInput")
    out_ext = nc.dram_tensor("out", [D, RPC], F32, kind="ExternalOutput")

    Exp = mybir.ActivationFunctionType.Exp

    with tile.TileContext(nc) as tc:
        with (
            tc.tile_pool(name="const", bufs=1) as constp,
            tc.tile_pool(name="xt", bufs=1) as xtp,
            tc.tile_pool(name="qk", bufs=2) as qkp,
            tc.tile_pool(name="pt", bufs=30) as ptp,
            tc.tile_pool(name="small", bufs=2) as smallp,
            tc.tile_pool(name="norm", bufs=2) as normp,
            tc.tile_pool(name="gather", bufs=1) as gap,
            tc.tile_pool(name="b1ps", bufs=4, space="PSUM") as b1_ps,
            tc.tile_pool(name="stps", bufs=2, space="PSUM") as st_ps,
            tc.tile_pool(name="dram", bufs=1, space="DRAM") as dramp,
        ):
            # ---- constants ----
            ones_sb = constp.tile([128, 64], BF16)
            nc.gpsimd.memset(ones_sb[:], 1.0)
            wq_sb = constp.tile([128, 8, 128], BF16)
            wk_sb = constp.tile([128, 8, 128], BF16)
            wv_sb = constp.tile([128, 8, 128], BF16)
            wo_sb = constp.tile([128, 8, D], BF16)
            mk_sb = constp.tile([128, 4, 512], BF16)
            eye_sb = constp.tile([128, 128], BF16)
            nc.sync.dma_start(wq_sb[:], wq_ext.rearrange("(c p) m -> p c m", p=128))
            nc.sync.dma_start(wk_sb[:], wk_ext.rearrange("(c p) m -> p c m", p=128))
            nc.sync.dma_start(wv_sb[:], wv_ext.rearrange("(c p) m -> p c m", p=128))
            nc.sync.dma_start(mk_sb[:], mk_ext.rearrange("j p f -> p j f"))
            nc.sync.dma_start(eye_sb[:], eye_ext[:])

            a2a_inA = dramp.tile([NCORES, 64, HPC, 512], BF16)
            a2a_outA = dramp.tile([NCORES, 64, HPC, 512], BF16)
            a2a_inB = dramp.tile([NCORES, 64, HPC, 512], BF16)
            a2a_outB = dramp.tile([NCORES, 64, HPC, 512], BF16)

            # ---------- per-batch stage emitters (software pipeline) ----------
            tiles = {}


            def qkv_load(b):
                xts = []
                for c in range(8):
                    xt = xtp.tile([128, S], BF16, tag=f"xt{c}")
                    # quarter-column pieces so the first QKV matmuls can
                    # start as soon as their slice lands
                    for q4 in range(4):
                        nc.sync.dma_start(
                            xt[:, q4 * 512:(q4 + 1) * 512],
                            xt_ext[
                                c * 128:(c + 1) * 128,
                                b * S + q4 * 512:b * S + (q4 + 1) * 512,
                            ],
                        )
                    xts.append(xt)
                va_b = qkp.tile([128, 16, 130], BF16, tag="va", name=f"va{b}")
                nc.gpsimd.memset(va_b[:, :, 64], 1.0)
                nc.gpsimd.memset(va_b[:, :, 129], 1.0)
                tiles[b] = {
                    "xts": xts,
                    "qt": qkp.tile([128, S], BF16, tag="qt", name=f"qt{b}"),
                    "kt": qkp.tile([128, S], BF16, tag="kt", name=f"kt{b}"),
                    "vt": qkp.tile([128, S], BF16, tag="vt", name=f"vt{b}", bufs=1),
                    "va": va_b,
                }

            def qkv_fill_nt(b, which, nt):
                # one 512-column slab of a projection: 8 accumulating matmuls
                wsb = {"qt": wq_sb, "kt": wk_sb, "vt": wv_sb}[which]
                dst = tiles[b][which]
                xts = tiles[b]["xts"]
                ps = b1_ps.tile([128, 512], F32, tag="b1")
                for c in range(8):
                    nc.tensor.matmul(
                        ps[:],
                        wsb[:, c, :],
                        xts[c][:, nt * 512:(nt + 1) * 512],
                        start=(c == 0),
                        stop=(c == 7),
                    )
                nc.vector.tensor_copy(dst[:, nt * 512:(nt + 1) * 512], ps[:])

            def v_trans_grp(b, grp):
                # V natural (seq rows on partitions) via PE transpose,
                # ones-augmented: [:, cb, 0:64]=V_h0, 64=1, 65:129=V_h1, 129=1
                va_b = tiles[b]["va"]
                vt_b = tiles[b]["vt"]
                tps = b1_ps.tile([128, 512], BF16, tag="b1")
                for k4 in range(4):
                    cb = grp * 4 + k4
                    nc.tensor.transpose(
                        tps[:, k4 * 128:(k4 + 1) * 128],
                        vt_b[:, cb * 128:(cb + 1) * 128],
                        eye_sb[:],
                    )
                psv = tps[:].rearrange("p (k d) -> p k d", k=4)
                nc.vector.tensor_copy(
                    va_b[:, grp * 4:(grp + 1) * 4, 0:64], psv[:, :, 0:64]
                )
                nc.vector.tensor_copy(
                    va_b[:, grp * 4:(grp + 1) * 4, 65:129], psv[:, :, 64:128]
                )

            def attn_head(b, h, gap_units=()):
                # scores^T (kt-stationary reused across q-chunks) -> exp ->
                # gpsimd 0/1 causal mask on diagonal chunk -> P^T@[V|1].
                # gap_units: next-batch QKV work, one unit emitted every other
                # s-tile so the PE FIFO has independent matmuls positioned
                # exactly where the exp-wait stalls are.
                gap_units = list(gap_units)
                hs = slice(h * 64, (h + 1) * 64)
                qt_b, kt_b = tiles[b]["qt"], tiles[b]["kt"]
                va_b = tiles[b]["va"]
                pts = []
                for ts in range(16):
                    if ts % 2 == 1 and gap_units:
                        gap_units.pop(0)()
                    tq0 = ts // 4
                    plist = []
                    p = 512 * tq0
                    while p < S:
                        pw = min(1024, S - p)
                        st = st_ps.tile([128, pw], F32, tag="st")
                        for i in range(pw // 512):
                            tq = (p + i * 512) // 512
                            nc.tensor.matmul(
                                st[:, i * 512:(i + 1) * 512],
                                kt_b[hs, ts * 128:(ts + 1) * 128],
                                qt_b[hs, tq * 512:(tq + 1) * 512],
                                start=True,
                                stop=True,
                            )
                        pt = ptp.tile([128, pw], BF16, tag="pt")
                        nc.scalar.activation(pt[:], st[:], Exp, scale=0.125)
                        if p == 512 * tq0:
                            nc.gpsimd.tensor_mul(
                                pt[:, 0:512], pt[:, 0:512], mk_sb[:, ts % 4, :]
                            )
                        plist.append((pt, p, pw))
                        p += pw
                    pts.append(plist)

                # PV even q-chunks first so the A-half A2A inputs complete
                # early; normalize per half. stg evictions go to ScalarE
                # (idle during these bursts — DVE is the hot engine here).
                stg = normp.tile([65, S], F32, tag="stg", name=f"stg{b}{h}")
                zrow = normp.tile([65, S], BF16, tag="zrow", name=f"zr{b}{h}")
                xo = normp.tile([64, S], BF16, tag="xo", name=f"xo{b}{h}")

                def pv_tq(tq):
                    aps = b1_ps.tile([65, 512], F32, tag="b1")
                    last = 4 * tq + 3
                    for ts in range(last + 1):
                        q = tq * 512
                        pt = off = None
                        for cand, p0, pw in pts[ts]:
                            if p0 <= q < p0 + pw:
                                pt, off = cand, q - p0
                                break
                        nc.tensor.matmul(
                            aps[:],
                            va_b[:, ts, h * 65:(h + 1) * 65],
                            pt[:, off:off + 512],
                            start=(ts == 0),
                            stop=(ts == last),
                        )
                    nc.scalar.copy(stg[:, tq * 512:(tq + 1) * 512], aps[:])

                def norm_tq(tq, dst):
                    sl = slice(tq * 512, (tq + 1) * 512)
                    with nc.allow_low_precision("bf16 softmax denominators"):
                        nc.vector.reciprocal(zrow[64:65, sl], stg[64:65, sl])
                    zps = b1_ps.tile([64, 512], F32, tag="b1")
                    nc.tensor.matmul(
                        zps[:], ones_sb[64:65, :], zrow[64:65, sl],
                        start=True, stop=True,
                    )
                    nc.vector.tensor_mul(xo[:, sl], stg[0:64, sl], zps[:])
                    nc.sync.dma_start(dst, xo[:, sl])

                pv_tq(0)
                pv_tq(2)
                # dest j=2b gets cols [0,1024) = tq0(A)+tq1(B);
                # dest j=2b+1 gets cols [1024,2048) = tq2(A)+tq3(B)
                norm_tq(0, a2a_inA[2 * b, :, h, :])
                norm_tq(2, a2a_inA[2 * b + 1, :, h, :])
                pv_tq(1)
                pv_tq(3)
                norm_tq(1, a2a_inB[2 * b, :, h, :])
                norm_tq(3, a2a_inB[2 * b + 1, :, h, :])
                if b == 0 and h == 0:
                    tap("stg", stg[:], [65, S], F32)
                    tap("zbb", xo[:], [64, S], BF16)
                return stg

            # ---------- pipelined emission: QKV(b+1) interleaves attn(b) ----
            qkv_load(0)
            # wo is only needed after the A2A — don't let its 2MB load
            # delay the first batch's xt pieces
            nc.sync.dma_start(wo_sb[:], wo_ext.rearrange("(c p) m -> p c m", p=128))
            for wch in ("qt", "kt", "vt"):
                for nt in range(4):
                    qkv_fill_nt(0, wch, nt)
            for grp in range(4):
                v_trans_grp(0, grp)
            tap("qt", tiles[0]["qt"][:], [128, S], BF16)
            tap("kt", tiles[0]["kt"][:], [128, S], BF16)
            tap("va", tiles[0]["va"][:], [128, 16, 130], BF16)

            for b in range(B):
                nb = b + 1
                if nb < B:
                    qkv_load(nb)
                    for nt in range(4):
                        qkv_fill_nt(nb, "qt", nt)
                attn_head(b, 0)
                if nb < B:
                    for nt in range(4):
                        qkv_fill_nt(nb, "kt", nt)
                    for nt in range(4):
                        qkv_fill_nt(nb, "vt", nt)
                        v_trans_grp(nb, nt)
                attn_head(b, 1)
                del tiles[b]

            # ---- AllToAll x2: redistribute head-columns -> row-slices ----
            # A covers each dest's cols [0,512), B covers [512,1024); the
            # output projection for r-slice 0 only needs A, so it overlaps B.
            ga = gap.tile([128, 8, RPC], BF16)
            # both triggers issued back-to-back so B isn't queued behind
            # A's gather DMAs on the gpsimd FIFO
            for a2a_in, a2a_out in ((a2a_inA, a2a_outA), (a2a_inB, a2a_outB)):
                nc.gpsimd.collective_compute(
                    "AllToAll",
                    mybir.AluOpType.bypass,
                    replica_groups=[list(range(NCORES))],
                    ins=[a2a_in[:].opt()],
                    outs=[a2a_out[:].opt()],
                )
            for rs, a2a_out in ((0, a2a_outA), (1, a2a_outB)):
                for i in range(NCORES):
                    nc.sync.dma_start(
                        ga[0:64, i, rs * 512:(rs + 1) * 512], a2a_out[i, :, 0, :]
                    )
                    nc.sync.dma_start(
                        ga[64:128, i, rs * 512:(rs + 1) * 512], a2a_out[i, :, 1, :]
                    )

            tap("ga", ga[:], [128, 8, RPC], BF16)

            # ---- output projection: y^T = wo^T-chunks @ attnT ----
            # warm-keeper matmuls: PE is otherwise idle during the A2A wait,
            # which would drop the HAM clock to 4/8 for the projection
            for rs in range(2):
                for mt in range(8):
                    yps = b1_ps.tile([128, 512], F32, tag="b1")
                    for u in range(8):
                        nc.tensor.matmul(
                            yps[:],
                            wo_sb[:, u, mt * 128:(mt + 1) * 128],
                            ga[:, u, rs * 512:(rs + 1) * 512],
                            start=(u == 0),
                            stop=(u == 7),
                        )
                    ys = smallp.tile([128, 512], F32, tag="ys")
                    nc.scalar.copy(ys[:], yps[:])
                    nc.sync.dma_start(
                        out_ext[mt * 128:(mt + 1) * 128, rs * 512:(rs + 1) * 512],
                        ys[:],
                    )

    nc.compile()
    return nc


def _prep(x, w_qkv, w_o):
    """Host-side shard prep: transpose/reshape/cast only."""
    bf = ml_dtypes.bfloat16
    xt = np.ascontiguousarray(x.reshape(N, D).T).astype(bf)
    woT = np.ascontiguousarray(w_o.T).astype(bf)
    masks = np.empty((4, 128, 512), dtype=np.float32)
    c = np.arange(128)[:, None]
    r = np.arange(512)[None, :]
    for j in range(4):
        masks[j] = np.where(c <= r - 128 * j, 1.0, 0.0)
    masks = masks.astype(bf)
    eye = np.eye(128, dtype=np.float32).astype(bf)

    in_maps = []
    for i in range(NCORES):
        h0, h1 = HPC * i, HPC * i + 1
        wq = np.concatenate([w_qkv[0, h0].T, w_qkv[0, h1].T], axis=1).astype(bf)
        wk = np.concatenate([w_qkv[1, h0].T, w_qkv[1, h1].T], axis=1).astype(bf)
        wv = np.concatenate([w_qkv[2, h0].T, w_qkv[2, h1].T], axis=1).astype(bf)
        in_maps.append(
            {
                "xt": xt,
                "wq": np.ascontiguousarray(wq),
                "wk": np.ascontiguousarray(wk),
                "wv": np.ascontiguousarray(wv),
                "wo": woT,
                "mk": masks,
                "eye": eye,
            }
        )
    return in_maps


def kernel(x, w_qkv, w_o, _trace=False):
    if "nc" not in _cache:
        _cache["nc"] = _build()
    nc = _cache["nc"]
    in_maps = _prep(
        np.asarray(x, np.float32),
        np.asarray(w_qkv, np.float32),
        np.asarray(w_o, np.float32),
    )
    res = run_bass_kernel_spmd(
        nc, in_maps, core_ids=list(range(NCORES)), trace=_trace
    )
    _cache["last"] = res
    y = np.concatenate(
        [np.asarray(res.results[i]["out"], np.float32).T for i in range(NCORES)],
        axis=0,
    )
    return y.reshape(B, S, D)



# revision 10
# speedup vs baseline: 1.4113x; 1.4113x over previous
"""Causal multi-head attention on 8 TRN2 NeuronCores.

Sharding: tensor-parallel over heads (16 heads -> 2 per core). Each core
computes Q/K/V projections for its 2 heads against the full (replicated)
input, runs causal attention for them, then a per-batch AllToAll
redistributes per-head attention numerators+denominators so each core owns
a 256-row slice of every batch and contracts the full head dimension for
the output projection.

All device layouts are "transposed" (feature dim on partitions):
  xT (D, B*S) -> QT/KT (128, S) per batch -> scores^T (k, q) -> P^T ->
  attnT (65, q) [unnormalized num + den row] -> A2A -> normalize ->
  y^T (D, 256) per batch.

Key structure vs a naive emission:
  - tq-major attention: for each 512-query chunk, stream key tiles,
    so P^T tiles die right after their PV matmul (low SBUF pressure).
  - h0/h1 score matmuls are K=64 and sit at partition bases 0/64, so
    back-to-back emission runs them concurrently in separate PE row
    groups; one merged exp covers both heads' PSUM banks.
  - scores computed only on the causal trapezoid; the ragged diagonal
    128-col block gets a single triangular 0/1 mask-mul on DVE; the
    below-diagonal prefix of diagonal P^T tiles is pre-zeroed once.
  - softmax normalization is deferred past the A2A: the PV matmul's
    ones-row gives the denominator for free; reciprocals for all 16
    heads are batched post-A2A ([16,256] DVE op) and broadcast via a
    tiny selection matmul, fused into the cast to bf16.
  - one AllToAll per batch, so collectives and the output projection
    overlap the next batch's attention; QKV(b+1) and outproj(b-1)
    matmuls are interleaved at the exp-wait positions of attn(b) to
    keep the PE busy (HAM stays at K=8/8).
"""

import os
import sys

for p in ("/opt/trn_rl_repo", "/root/.axon_site/_ro/trn_rl_repo"):
    if os.path.isdir(p) and p not in sys.path:
        sys.path.append(p)

import numpy as np
import ml_dtypes

import concourse.mybir as mybir
import concourse.tile as tile
from concourse import bacc
from concourse.bass_utils import run_bass_kernel_spmd

B, S, D, H, DK = 4, 2048, 1024, 16, 64
NCORES = 8
HPC = H // NCORES          # heads per core = 2
N = B * S                  # 8192 flattened rows
QPC = S // NCORES          # query rows per (core, batch) after A2A = 256
BF16 = mybir.dt.bfloat16
F32 = mybir.dt.float32

_cache = {}


def _build(dbg=False):
    nc = bacc.Bacc(None, target_bir_lowering=False, debug=False)

    dbg_taps = {}

    def tap(name, src_ap, shape, dtype):
        if not dbg:
            return
        t = nc.dram_tensor(f"dbg_{name}", shape, dtype, kind="ExternalOutput")
        nc.sync.dma_start(t[:], src_ap)
        dbg_taps[name] = t

    xt_ext = nc.dram_tensor("xt", [D, N], BF16, kind="ExternalInput")
    wq_ext = nc.dram_tensor("wq", [D, 128], BF16, kind="ExternalInput")
    wk_ext = nc.dram_tensor("wk", [D, 128], BF16, kind="ExternalInput")
    wv_ext = nc.dram_tensor("wv", [D, 128], BF16, kind="ExternalInput")
    wo_ext = nc.dram_tensor("wo", [D, D], BF16, kind="ExternalInput")
    tri_ext = nc.dram_tensor("tri", [128, 128], BF16, kind="ExternalInput")
    eye_ext = nc.dram_tensor("eye", [128, 128], BF16, kind="ExternalInput")
    bsel_ext = nc.dram_tensor("bsel", [16, 8, 128], BF16, kind="ExternalInput")
    out_ext = nc.dram_tensor("out", [D, B, QPC], F32, kind="ExternalOutput")

    Exp = mybir.ActivationFunctionType.Exp

    with tile.TileContext(nc) as tc:
        with (
            tc.tile_pool(name="const", bufs=1) as constp,
            tc.tile_pool(name="xt", bufs=2) as xtp,
            tc.tile_pool(name="qk", bufs=2) as qkp,
            tc.tile_pool(name="pt", bufs=6) as ptp,
            tc.tile_pool(name="xo", bufs=6) as xop,
            tc.tile_pool(name="gath", bufs=2) as gap,
            tc.tile_pool(name="ys", bufs=2) as ysp,
            tc.tile_pool(name="stps", bufs=2, space="PSUM") as st_ps,
            tc.tile_pool(name="apsps", bufs=2, space="PSUM") as aps_ps,
            tc.tile_pool(name="b1ps", bufs=2, space="PSUM") as b1_ps,
            tc.tile_pool(name="dram", bufs=1, space="DRAM") as dramp,
        ):
            # ---- constants ----
            wq_sb = constp.tile([128, 8, 128], BF16)
            wk_sb = constp.tile([128, 8, 128], BF16)
            wv_sb = constp.tile([128, 8, 128], BF16)
            wo_sb = constp.tile([128, 8, D], BF16)
            tri_sb = constp.tile([128, 128], BF16)
            eye_sb = constp.tile([128, 128], BF16)
            bsel_sb = constp.tile([16, 8, 128], BF16)
            nc.sync.dma_start(wq_sb[:], wq_ext.rearrange("(c p) m -> p c m", p=128))
            nc.sync.dma_start(wk_sb[:], wk_ext.rearrange("(c p) m -> p c m", p=128))
            nc.sync.dma_start(wv_sb[:], wv_ext.rearrange("(c p) m -> p c m", p=128))
            nc.sync.dma_start(tri_sb[:], tri_ext[:])
            nc.sync.dma_start(eye_sb[:], eye_ext[:])
            nc.sync.dma_start(bsel_sb[:], bsel_ext[:])

            # diagonal P^T tiles with persistent zero prefixes; class c has
            # cols [0,128c) and [512,512+128c) permanently zero
            ptd = {}
            for c in (1, 2, 3):
                for par in (0, 1):
                    t = constp.tile([128, 1024], BF16, name=f"ptd{c}_{par}")
                    nc.gpsimd.memset(t[:, 0:128 * c], 0.0)
                    nc.gpsimd.memset(t[:, 512:512 + 128 * c], 0.0)
                    ptd[(c, par)] = t

            a2a_in = [dramp.tile([NCORES, HPC, 65, QPC], BF16, name=f"a2ai{b}")
                      for b in range(B)]
            a2a_out = [dramp.tile([NCORES, HPC, 65, QPC], BF16, name=f"a2ao{b}")
                       for b in range(B)]

            tiles = {}

            def qkv_load(b):
                xts = []
                for c in range(8):
                    xt = xtp.tile([128, S], BF16, tag=f"xt{c}")
                    # quarter-column pieces so the first QKV matmuls can
                    # start as soon as their slice lands
                    for q4 in range(4):
                        nc.sync.dma_start(
                            xt[:, q4 * 512:(q4 + 1) * 512],
                            xt_ext[
                                c * 128:(c + 1) * 128,
                                b * S + q4 * 512:b * S + (q4 + 1) * 512,
                            ],
                        )
                    xts.append(xt)
                va_b = qkp.tile([128, 16, 130], BF16, tag="va", name=f"va{b}")
                nc.gpsimd.memset(va_b[:, :, 64], 1.0)
                nc.gpsimd.memset(va_b[:, :, 129], 1.0)
                tiles[b] = {
                    "xts": xts,
                    "qt": qkp.tile([128, S], BF16, tag="qt", name=f"qt{b}"),
                    "kt": qkp.tile([128, S], BF16, tag="kt", name=f"kt{b}"),
                    "vt": qkp.tile([128, S], BF16, tag="vt", name=f"vt{b}", bufs=1),
                    "va": va_b,
                }

            def qkv_fill_nt(b, which, nt):
                # one 512-column slab of a projection: 8 accumulating matmuls
                wsb = {"qt": wq_sb, "kt": wk_sb, "vt": wv_sb}[which]
                dst = tiles[b][which]
                xts = tiles[b]["xts"]
                ps = b1_ps.tile([128, 512], F32, tag="b1")
                for c in range(8):
                    nc.tensor.matmul(
                        ps[:],
                        wsb[:, c, :],
                        xts[c][:, nt * 512:(nt + 1) * 512],
                        start=(c == 0),
                        stop=(c == 7),
                    )
                nc.vector.tensor_copy(dst[:, nt * 512:(nt + 1) * 512], ps[:])

            def v_trans_grp(b, grp):
                # V natural (seq rows on partitions) via PE transpose,
                # ones-augmented: [:, cb, 0:64]=V_h0, 64=1, 65:129=V_h1, 129=1
                va_b = tiles[b]["va"]
                vt_b = tiles[b]["vt"]
                tps = b1_ps.tile([128, 512], BF16, tag="b1")
                for k4 in range(4):
                    cb = grp * 4 + k4
                    nc.tensor.transpose(
                        tps[:, k4 * 128:(k4 + 1) * 128],
                        vt_b[:, cb * 128:(cb + 1) * 128],
                        eye_sb[:],
                    )
                psv = tps[:].rearrange("p (k d) -> p k d", k=4)
                nc.vector.tensor_copy(
                    va_b[:, grp * 4:(grp + 1) * 4, 0:64], psv[:, :, 0:64]
                )
                nc.vector.tensor_copy(
                    va_b[:, grp * 4:(grp + 1) * 4, 65:129], psv[:, :, 64:128]
                )

            # ---------- attention for batch b, both heads, tq-major ----------
            def attn_batch(b, units):
                units = list(units)
                step = [0]
                qt_b, kt_b = tiles[b]["qt"], tiles[b]["kt"]
                va_b = tiles[b]["va"]

                def scores_pair(tq, ts):
                    # one key-tile's scores for both heads (concurrent PE
                    # row groups); returns (st, pt, off) for the pair
                    c = ts - 4 * tq
                    st = st_ps.tile([128, 1024], F32, tag="st")
                    if c < 0:
                        off = 0
                        pt = ptp.tile([128, 1024], BF16, tag="pt")
                    else:
                        off = 128 * c
                        pt = ptd[(c, tq % 2)] if c > 0 else ptp.tile(
                            [128, 1024], BF16, tag="pt"
                        )
                    q0 = 512 * tq + off
                    nw = 512 - off
                    for h in (0, 1):
                        hs = slice(h * 64, (h + 1) * 64)
                        nc.tensor.matmul(
                            st[:, 512 * h + off:512 * h + 512],
                            kt_b[hs, ts * 128:(ts + 1) * 128],
                            qt_b[hs, q0:q0 + nw],
                            start=True,
                            stop=True,
                        )
                    # exp -> P^T (bf16); merged across both heads when the
                    # written span is contiguous (off == 0)
                    if off == 0:
                        nc.scalar.activation(pt[:], st[:], Exp, scale=0.125)
                    else:
                        for h in (0, 1):
                            sl = slice(512 * h + off, 512 * h + 512)
                            nc.scalar.activation(pt[:, sl], st[:, sl], Exp,
                                                 scale=0.125)
                    if c >= 0:
                        # triangular causal mask on the 128-wide diagonal block
                        for h in (0, 1):
                            sl = slice(512 * h + off, 512 * h + off + 128)
                            nc.vector.tensor_mul(pt[:, sl], pt[:, sl], tri_sb[:])
                    return pt

                def emit_unit():
                    # the first few steps stay unit-free so early fillers
                    # don't head-of-line-block the PE FIFO on input DMAs
                    step[0] += 1
                    if units and step[0] > 3:
                        units.pop(0)()

                for tq in range(4):
                    last = 4 * tq + 3
                    aps = {h: aps_ps.tile([65, 512], F32, tag="aps",
                                          name=f"aps{b}{tq}{h}")
                           for h in (0, 1)}
                    pts = {}
                    pts[0] = scores_pair(tq, 0)
                    for ts in range(last + 1):
                        if ts + 1 <= last:
                            pts[ts + 1] = scores_pair(tq, ts + 1)
                        emit_unit()
                        pt = pts.pop(ts)
                        for h in (0, 1):
                            nc.tensor.matmul(
                                aps[h][:],
                                va_b[:, ts, h * 65:(h + 1) * 65],
                                pt[:, 512 * h:512 * h + 512],
                                start=(ts == 0),
                                stop=(ts == last),
                            )
                    # evict num+den, ship to the A2A buffer
                    for h in (0, 1):
                        xo = xop.tile([65, 512], BF16, tag="xo")
                        nc.vector.tensor_copy(xo[:], aps[h][:])
                        for j in (0, 1):
                            nc.sync.dma_start(
                                a2a_in[b][2 * tq + j, h, :, :],
                                xo[:, j * 256:(j + 1) * 256],
                            )
                        if b == 0 and tq == 0 and h == 0:
                            tap("xo", xo[:], [65, 512], BF16)
                for u in units:
                    u()

            # ---------- post-A2A: normalize + output projection ----------
            def outproj_prep(b):
                ga = gap.tile([128, 8, QPC], BF16, tag="ga", name=f"ga{b}")
                db = gap.tile([16, QPC], BF16, tag="db", name=f"db{b}")
                for h in (0, 1):
                    nc.sync.dma_start(
                        ga[64 * h:64 * h + 64, :, :],
                        a2a_out[b][:, h, 0:64, :].rearrange("u k q -> k u q"),
                    )
                    nc.sync.dma_start(
                        db[8 * h:8 * h + 8, :],
                        a2a_out[b][:, h, 64, :],
                    )
                rb = gap.tile([16, QPC], BF16, tag="rb", name=f"rb{b}")
                with nc.allow_low_precision("bf16 softmax denominators"):
                    nc.vector.reciprocal(rb[:], db[:])
                if b == 0:
                    tap("ga", ga[:], [128, 8, QPC], BF16)
                    tap("db", db[:], [16, QPC], BF16)
                gs = gap.tile([128, 8, QPC], BF16, tag="gs", name=f"gs{b}")
                tiles[f"oj{b}"] = (ga, rb, gs)

            def outproj_norm(b, u):
                # broadcast 1/den over the 64 head dims via selection matmul,
                # then scale+cast the numerators
                ga, rb, gs = tiles[f"oj{b}"]
                bps = b1_ps.tile([128, QPC], F32, tag="b1")
                nc.tensor.matmul(bps[:], bsel_sb[:, u, :], rb[:],
                                 start=True, stop=True)
                nc.vector.tensor_mul(gs[:, u, :], ga[:, u, :], bps[:])
                if b == 0 and u == 0:
                    tap("gs0", gs[:, 0, :], [128, QPC], BF16)

            def outproj_mt(b, mt):
                _, _, gs = tiles[f"oj{b}"]
                yps = b1_ps.tile([128, QPC], F32, tag="b1")
                for u in range(8):
                    nc.tensor.matmul(
                        yps[:],
                        wo_sb[:, u, mt * 128:(mt + 1) * 128],
                        gs[:, u, :],
                        start=(u == 0),
                        stop=(u == 7),
                    )
                ys = ysp.tile([128, QPC], F32, tag="ys")
                nc.vector.tensor_copy(ys[:], yps[:])
                nc.sync.dma_start(out_ext[mt * 128:(mt + 1) * 128, b, :], ys[:])

            # ---------- pipelined emission ----------
            qkv_load(0)
            # wo is only needed for the output projection — don't let its
            # 2MB load delay the first batch's xt pieces
            nc.sync.dma_start(wo_sb[:], wo_ext.rearrange("(c p) m -> p c m", p=128))
            for wch in ("qt", "kt", "vt"):
                for nt in range(4):
                    qkv_fill_nt(0, wch, nt)
            for grp in range(4):
                v_trans_grp(0, grp)
            tap("qt", tiles[0]["qt"][:], [128, S], BF16)
            tap("kt", tiles[0]["kt"][:], [128, S], BF16)
            tap("va", tiles[0]["va"][:], [128, 16, 130], BF16)

            for b in range(B):
                nb = b + 1
                qk_units, oj_units = [], []
                if nb < B:
                    qkv_load(nb)
                    for nt in range(4):
                        qk_units.append(
                            lambda b=nb, nt=nt: qkv_fill_nt(b, "qt", nt))
                    for nt in range(4):
                        qk_units.append(
                            lambda b=nb, nt=nt: qkv_fill_nt(b, "kt", nt))
                    for nt in range(4):
                        qk_units.append(
                            lambda b=nb, nt=nt: qkv_fill_nt(b, "vt", nt))
                        qk_units.append(lambda b=nb, g=nt: v_trans_grp(b, g))
                if b >= 1:
                    pb = b - 1
                    # gathers + reciprocal early (they only occupy DMA/DVE);
                    # broadcast + projection PE work spreads out later, after
                    # the previous batch's collective has certainly landed
                    oj_units.append(lambda pb=pb: outproj_prep(pb))
                    for u in range(8):
                        oj_units.append(lambda pb=pb, u=u: outproj_norm(pb, u))
                    for mt in range(8):
                        oj_units.append(lambda pb=pb, mt=mt: outproj_mt(pb, mt))
                # round-robin merge, 2 qkv : 1 outproj; qkv fills lead since
                # their inputs stream in first
                units = []
                qi = oi = 0
                while qi < len(qk_units) and oi < len(oj_units):
                    units.append(qk_units[qi]); qi += 1
                    if qi < len(qk_units):
                        units.append(qk_units[qi]); qi += 1
                    units.append(oj_units[oi]); oi += 1
                units.extend(qk_units[qi:])
                units.extend(oj_units[oi:])
                attn_batch(b, units)
                nc.gpsimd.collective_compute(
                    "AllToAll",
                    mybir.AluOpType.bypass,
                    replica_groups=[list(range(NCORES))],
                    ins=[a2a_in[b][:].opt()],
                    outs=[a2a_out[b][:].opt()],
                )
                del tiles[b]

            # tail: last batch's projection
            outproj_prep(B - 1)
            for u in range(8):
                outproj_norm(B - 1, u)
            for mt in range(8):
                outproj_mt(B - 1, mt)

    nc.compile()
    return nc


def _prep(x, w_qkv, w_o):
    """Host-side shard prep: transpose/reshape/cast only."""
    bf = ml_dtypes.bfloat16
    xt = np.ascontiguousarray(x.reshape(N, D).T).astype(bf)
    woT = np.ascontiguousarray(w_o.T).astype(bf)
    tri = np.triu(np.ones((128, 128), dtype=np.float32)).astype(bf)
    eye = np.eye(128, dtype=np.float32).astype(bf)
    # den-row staging layout: db[u + 8h] holds head (2u+h)'s denominators
    bsel = np.zeros((16, 8, 128), dtype=np.float32)
    for u in range(8):
        bsel[u, u, 0:64] = 1.0
        bsel[u + 8, u, 64:128] = 1.0
    bsel = bsel.astype(bf)

    in_maps = []
    for i in range(NCORES):
        h0, h1 = HPC * i, HPC * i + 1
        wq = np.concatenate([w_qkv[0, h0].T, w_qkv[0, h1].T], axis=1).astype(bf)
        wk = np.concatenate([w_qkv[1, h0].T, w_qkv[1, h1].T], axis=1).astype(bf)
        wv = np.concatenate([w_qkv[2, h0].T, w_qkv[2, h1].T], axis=1).astype(bf)
        in_maps.append(
            {
                "xt": xt,
                "wq": np.ascontiguousarray(wq),
                "wk": np.ascontiguousarray(wk),
                "wv": np.ascontiguousarray(wv),
                "wo": woT,
                "tri": tri,
                "eye": eye,
                "bsel": bsel,
            }
        )
    return in_maps


def kernel(x, w_qkv, w_o, _trace=False):
    if "nc" not in _cache:
        _cache["nc"] = _build()
    nc = _cache["nc"]
    in_maps = _prep(
        np.asarray(x, np.float32),
        np.asarray(w_qkv, np.float32),
        np.asarray(w_o, np.float32),
    )
    res = run_bass_kernel_spmd(
        nc, in_maps, core_ids=list(range(NCORES)), trace=_trace
    )
    _cache["last"] = res
    # res[j]["out"]: [D, B, QPC] f32; y[b, 256j:256(j+1), :] = out[:, b, :].T
    y = np.empty((B, S, D), np.float32)
    for j in range(NCORES):
        o = np.asarray(res.results[j]["out"], np.float32)
        for b in range(B):
            y[b, QPC * j:QPC * (j + 1), :] = o[:, b, :].T
    return y
